# revision 20
# baseline (speedup 1.0000x reference)
import os
import sys
from contextlib import ExitStack

import numpy as np

for _p in ("/opt/trn_rl_repo", "/root/.axon_site/_ro/trn_rl_repo"):
    if os.path.isdir(_p) and _p not in sys.path:
        sys.path.insert(0, _p)

import concourse.bass as bass
from concourse import mybir
from concourse.bass_utils import run_bass_kernel_spmd

B, D, C = 32768, 1024, 256
M_CORES = 8
BS = B // M_CORES
P = 128
GQ = 4
N_GROUPS = BS // (P * GQ)
N_SUB = N_GROUPS * GQ
WEIGHT = 0.0005
EPS = 1e-12

F32 = mybir.dt.float32
F32R = mybir.dt.float32r
BF16 = mybir.dt.bfloat16

XSLOTS = 7
XSS = 4
OHB = 8
OHF = 4
N_WARM = 16


def _sub_kind(t, a):
    return a == 3 and t % 2 == 0


def _ssq_on_dve(t, a):
    return a == 3


def build_nc(bs=BS):
    Sq = mybir.ActivationFunctionType.Square
    Sqrt = mybir.ActivationFunctionType.Sqrt
    CopyF = mybir.ActivationFunctionType.Copy

    LASTG = N_GROUPS - 1
    FINE_T = (0, 1, LASTG)

    def ssq_dve(t, a):
        return a == 3 or (t == LASTG and a == 2)

    dma_seq = [("aux",)]
    dma_seq += [("x", 0, a) for a in range(GQ)]
    dma_seq += [("x", 1, 0), ("x", 1, 1), ("lab",), ("x", 1, 2), ("x", 1, 3)]
    for t in range(2, N_GROUPS):
        if t in FINE_T:
            dma_seq += [("x", t, a) for a in range(GQ)]
        else:
            dma_seq += [("xg", t)]
    dma_idx = {}
    for i, e in enumerate(dma_seq):
        dma_idx[e] = (i + 1) * 16

    def x_wait(t, a):
        if t in FINE_T:
            return dma_idx[("x", t, a)]
        return dma_idx[("xg", t)]

    act_seq = []
    for t in range(N_GROUPS):
        if t == LASTG:
            for a in range(GQ):
                if not ssq_dve(t, a):
                    act_seq.append(("sq", t, a))
                    act_seq.append(("nrm", t, a))
            for a in range(GQ):
                if ssq_dve(t, a):
                    act_seq.append(("nrm", t, a))
        else:
            for a in range(GQ):
                if not ssq_dve(t, a):
                    act_seq.append(("sq", t, a))
            act_seq.append(("nrm", t))
    act_seq.append(("copyB",))
    act_idx = {e: i + 1 for i, e in enumerate(act_seq)}

    def nrm_wait(t, a):
        if t == LASTG:
            return act_idx[("nrm", t, a)]
        return act_idx[("nrm", t)]

    dve_seq = []
    for t in range(N_GROUPS):
        if t >= 1:
            tg = t - 1
            dve_seq.append(("recip", tg))
            dve_seq += [("oh", tg, a) for a in range(GQ)]
        if t == LASTG:
            dve_seq.append(("stt", t, 2))
            dve_seq.append(("stt", t, 3))
        else:
            dve_seq.append(("stt", t, 3))
        if t == 5:
            dve_seq.append(("copyA", 0))
            dve_seq.append(("copyA", 1))
    for a in range(GQ):
        dve_seq.append(("recip", LASTG, a))
        dve_seq.append(("oh", LASTG, a))
    dve_seq.append(("copyB",))
    dve_idx = {e: i + 1 for i, e in enumerate(dve_seq)}

    def oh_wait(t, a):
        if t == LASTG:
            return dve_idx[("oh", t, a)]
        return dve_idx[("oh", t, a)]

    def stt_wait(t, a):
        return dve_idx[("stt", t, a)]

    nc = bass.Bass()
    x = nc.declare_dram_parameter("x", [bs, D], F32R, isOutput=False)
    lab = nc.declare_dram_parameter("labf", [P, N_SUB], F32, isOutput=False)
    aux = nc.declare_dram_parameter("aux", [P, C + 2], F32R, isOutput=False)
    sumsA = nc.declare_dram_parameter("sumsA", [C, D], BF16, isOutput=True)
    sumsB = nc.declare_dram_parameter("sumsB", [C, D], BF16, isOutput=True)

    with ExitStack() as stk:
        en = stk.enter_context
        xt = en(nc.sbuf_tensor([P, XSLOTS, GQ, D], F32R))
        sqscr = en(nc.sbuf_tensor([P, D], BF16))
        vscr = en(nc.sbuf_tensor([P, D], BF16))
        auxs = en(nc.sbuf_tensor([P, C + 2], F32R))
        labf = en(nc.sbuf_tensor([P, N_SUB], F32))
        ssq = en(nc.sbuf_tensor([P, N_SUB], F32))
        nrm = en(nc.sbuf_tensor([P, N_SUB], F32))
        rr = en(nc.sbuf_tensor([P, N_SUB], F32))
        ohr = en(nc.sbuf_tensor([P, OHB, C], F32R))
        outA = en(nc.sbuf_tensor([P, 2, D], BF16))
        outB = en(nc.sbuf_tensor([P, 2, D], BF16))
        dum = en(nc.sbuf_tensor([P, 2], F32))
        ps = {}
        for h in range(2):
            for mi in range(2):
                ps[(h, mi)] = en(nc.psum_tensor(f"ps_{h}{mi}", [P, D], F32))

        s_in = en(nc.semaphore("s_in"))
        s_act = en(nc.semaphore("s_act"))
        s_dve = en(nc.semaphore("s_dve"))
        s_pe_mm = en(nc.semaphore("s_pe_mm"))
        s_out = en(nc.semaphore("s_out"))
        block = en(nc.Block(no_gpsimd_drain=True))

        def x_src(t):
            return x[t * P * GQ : (t + 1) * P * GQ, :].rearrange(
                "(p g) d -> p g d", p=P
            )

        @block.sync
        def _(sync):
            sync.dma_start(out=auxs[:, :], in_=aux[:, :]).then_inc(s_in, 16)
            for e in dma_seq[1:]:
                if e[0] == "lab":
                    sync.dma_start(out=labf[:, :], in_=lab[:, :]).then_inc(
                        s_in, 16
                    )
                elif e[0] == "x":
                    _, t, a = e
                    if t >= XSLOTS:
                        sync.wait_ge(s_pe_mm, GQ * (t - XSLOTS + 1))
                    sync.dma_start(
                        out=xt[:, t % XSLOTS, a, :], in_=x_src(t)[:, a, :]
                    ).then_inc(s_in, 16)
                else:
                    _, t = e
                    if t >= XSLOTS:
                        sync.wait_ge(s_pe_mm, GQ * (t - XSLOTS + 1))
                    sync.dma_start(
                        out=xt[:, t % XSLOTS, :, :], in_=x_src(t)[:, :, :]
                    ).then_inc(s_in, 16)
            sync.wait_ge(s_dve, dve_idx[("copyA", 0)])
            sync.dma_start(out=sumsA[0:128, :], in_=outA[:, 0, :]).then_inc(
                s_out, 16
            )
            sync.wait_ge(s_dve, dve_idx[("copyA", 1)])
            sync.dma_start(out=sumsA[128:256, :], in_=outA[:, 1, :]).then_inc(
                s_out, 16
            )
            sync.wait_ge(s_act, act_idx[("copyB",)])
            sync.dma_start(out=sumsB[0:128, :], in_=outB[:, 0, :]).then_inc(
                s_out, 16
            )
            sync.wait_ge(s_dve, dve_idx[("copyB",)])
            sync.dma_start(out=sumsB[128:256, :], in_=outB[:, 1, :]).then_inc(
                s_out, 16
            )
            sync.wait_ge(s_out, 64)

        @block.scalar
        def _(scalar):
            scalar.activation(dum[:, 0:1], dum[:, 1:2], Sq, bias=0.0)
            scalar.activation(dum[:, 0:1], dum[:, 1:2], Sqrt, bias=0.0)
            scalar.wait_ge(s_in, dma_idx[("aux",)])
            eps_bias = auxs.bitcast(F32)[:, C + 1 : C + 2]
            n_sq = 0
            for e in act_seq:
                if e[0] == "sq":
                    _, t, a = e
                    k = t * GQ + a
                    scalar.wait_ge(s_in, x_wait(t, a))
                    scalar.activation(
                        sqscr[:, :],
                        xt.bitcast(F32)[:, t % XSLOTS, a, :],
                        Sq,
                        bias=0.0,
                        accum_out=ssq[:, k : k + 1],
                    ).then_inc(s_act, 1)
                    n_sq = act_idx[e]
                elif e[0] == "nrm":
                    if len(e) == 3:
                        _, t, a = e
                        if ssq_dve(t, a):
                            scalar.wait_ge(s_dve, stt_wait(t, a))
                        else:
                            scalar.wait_ge(s_act, n_sq)
                        k = t * GQ + a
                        scalar.activation(
                            nrm[:, k : k + 1], ssq[:, k : k + 1], Sqrt,
                            bias=eps_bias,
                        ).then_inc(s_act, 1)
                    else:
                        _, t = e
                        scalar.wait_ge(s_act, n_sq)
                        scalar.wait_ge(s_dve, stt_wait(t, 3))
                        scalar.activation(
                            nrm[:, t * GQ : (t + 1) * GQ],
                            ssq[:, t * GQ : (t + 1) * GQ],
                            Sqrt,
                            bias=eps_bias,
                        ).then_inc(s_act, 1)
                else:
                    scalar.wait_ge(s_pe_mm, N_SUB)
                    scalar.activation(
                        outB[:, 0, :], ps[(1, 0)][:, :], CopyF
                    ).then_inc(s_act, 1)

        @block.vector
        def _(vector):
            vector.wait_ge(s_in, dma_idx[("lab",)])
            for e in dve_seq:
                if e[0] == "stt":
                    _, t, a = e
                    vector.wait_ge(s_in, x_wait(t, a))
                    k = t * GQ + a
                    xv = xt.bitcast(F32)[:, t % XSLOTS, a, :]
                    vector.scalar_tensor_tensor(
                        vscr[:, :], xv, 1.0, xv,
                        mybir.AluOpType.mult, mybir.AluOpType.mult,
                        accum_out=ssq[:, k : k + 1],
                    ).then_inc(s_dve, 1)
                elif e[0] == "recip":
                    if len(e) == 3:
                        _, tg, a = e
                        vector.wait_ge(s_act, nrm_wait(tg, a))
                        k = tg * GQ + a
                        vector.reciprocal(
                            rr[:, k : k + 1], nrm[:, k : k + 1]
                        ).then_inc(s_dve, 1)
                    else:
                        _, tg = e
                        vector.wait_ge(s_act, nrm_wait(tg, 0))
                        sl = slice(tg * GQ, (tg + 1) * GQ)
                        vector.reciprocal(rr[:, sl], nrm[:, sl]).then_inc(
                            s_dve, 1
                        )
                elif e[0] == "oh":
                    _, tg, a = e
                    k = tg * GQ + a
                    if a == 0 or tg == LASTG:
                        vector.wait_ge(s_dve, dve_idx[("recip", tg)
                                       if tg != LASTG else ("recip", tg, a)])
                    if k >= OHB and (a == 0 or tg == LASTG):
                        kl = tg * GQ + (GQ - 1 if tg != LASTG else a)
                        vector.wait_ge(s_pe_mm, kl - OHB + 1)
                    vector.tensor_scalar(
                        ohr[:, k % OHB, :],
                        auxs.bitcast(F32)[:, 0:C],
                        labf[:, k : k + 1],
                        rr[:, k : k + 1],
                        mybir.AluOpType.is_equal,
                        mybir.AluOpType.mult,
                    ).then_inc(s_dve, 1)
                elif e[0] == "copyA":
                    _, mi = e
                    vector.wait_ge(s_pe_mm, 16)
                    vector.tensor_copy(
                        outA[:, mi, :], ps[(0, mi)][:, :]
                    ).then_inc(s_dve, 1)
                else:
                    vector.wait_ge(s_pe_mm, N_SUB)
                    vector.tensor_copy(outB[:, 1, :], ps[(1, 1)][:, :]).then_inc(
                        s_dve, 1
                    )

        @block.tensor
        def _(tensor):
            tensor.wait_ge(s_in, dma_idx[("aux",)])
            for _ in range(N_WARM):
                tensor.matmul(
                    ps[(1, 0)][:, 0:256],
                    auxs[:, 0:128],
                    auxs[:, 0:256],
                    start=True,
                    stop=True,
                )
            for t in range(N_GROUPS):
                for a in range(GQ):
                    k = t * GQ + a
                    h = 0 if t < N_GROUPS // 2 else 1
                    first = k % 16 == 0
                    last = k % 16 == 15
                    tensor.wait_ge(s_dve, oh_wait(t, a))
                    tensor.wait_ge(s_in, x_wait(t, a))
                    w = ohr[:, k % OHB, :]
                    mv = xt[:, t % XSLOTS, a, :]
                    i = None
                    for mi in range(2):
                        for ni in range(2):
                            i = tensor.matmul(
                                ps[(h, mi)][:, ni * 512 : (ni + 1) * 512],
                                w[:, mi * 128 : (mi + 1) * 128],
                                mv[:, ni * 512 : (ni + 1) * 512],
                                start=first,
                                stop=last,
                            )
                    i.then_inc(s_pe_mm, 1)

    return nc


def _norm_rows(x):
    x = x.astype(np.float64)
    n = np.sqrt((x * x).sum(axis=-1, keepdims=True))
    return x / np.maximum(n, EPS)


def _host_finish(feats, labels, S):
    b, d = feats.shape
    counts = np.bincount(labels, minlength=C)
    n = counts.astype(np.float64)
    mask = n > 1.0
    normS2 = (S * S).sum(axis=1)
    term1 = float(((n - normS2 / np.maximum(n, 1.0)) * mask).sum())

    nc_of_row = counts[labels]
    rows = np.nonzero(np.arange(b) < nc_of_row)[0]
    corr = 0.0
    if rows.size:
        order = np.argsort(labels, kind="stable")
        cls_sorted = labels[order]
        starts = np.searchsorted(cls_sorted, np.arange(C))
        need = set()
        for i in rows:
            c = int(labels[i])
            if counts[c] <= 1:
                continue
            k = int(order[starts[c] + i])
            need.add(int(i))
            need.add(k)
        need = sorted(need)
        fcache = {i: _norm_rows(feats[i]) for i in need}
        for i in rows:
            c = int(labels[i])
            n_c = float(counts[c])
            if n_c <= 1.0:
                continue
            k = int(order[starts[c] + i])
            f_i = fcache[int(i)]
            f_k = fcache[k]
            Sc = S[c]
            c_simple = Sc / n_c
            c_true = (Sc - f_k) / (n_c - 1.0)
            d_true = float(((f_i - c_true) ** 2).sum())
            d_simple = float(((f_i - c_simple) ** 2).sum())
            corr += d_true - d_simple

    total = term1 + corr
    return np.array(WEIGHT * total / (b * d), dtype=np.float32)


_nc_cache = None

TRACE = False
LAST_RESULTS = None


def _round_fp32r(a):
    b = np.ascontiguousarray(a, dtype=np.float32).view(np.uint32)
    return ((b + 0x800) & 0xFFFFF000).view(np.float32)


def _aux_input():
    a = np.zeros((P, C + 2), dtype=np.float32)
    a[:, :C] = np.arange(C, dtype=np.float32)[None, :]
    a[:, C + 1] = 1e-20
    return _round_fp32r(a)


def kernel(features, labels):
    global _nc_cache, LAST_RESULTS
    feats = np.ascontiguousarray(np.asarray(features, dtype=np.float32))
    feats_r = _round_fp32r(feats)
    labs = np.ascontiguousarray(np.asarray(labels, dtype=np.int32))
    assert feats.shape == (B, D) and labs.shape == (B,)
    labs_f = labs.astype(np.float32)
    aux = _aux_input()
    if _nc_cache is None:
        _nc_cache = build_nc()
    in_maps = []
    for m in range(M_CORES):
        lf = labs_f[m * BS : (m + 1) * BS]
        lf = np.ascontiguousarray(
            lf.reshape(N_GROUPS, P, GQ).transpose(1, 0, 2).reshape(P, N_SUB)
        )
        in_maps.append(
            {"x": feats_r[m * BS : (m + 1) * BS], "labf": lf, "aux": aux}
        )
    res = run_bass_kernel_spmd(
        _nc_cache, in_maps, core_ids=list(range(M_CORES)), trace=TRACE
    )
    LAST_RESULTS = res
    S = np.zeros((C, D), np.float64)
    for r in res.results:
        S += r["sumsA"].astype(np.float64)
        S += r["sumsB"].astype(np.float64)
    return _host_finish(feats, labs, S)


# revision 23
# speedup vs baseline: 1.0578x; 1.0578x over previous
import os
import sys
from contextlib import ExitStack

import numpy as np

for _p in ("/opt/trn_rl_repo", "/root/.axon_site/_ro/trn_rl_repo"):
    if os.path.isdir(_p) and _p not in sys.path:
        sys.path.insert(0, _p)

import concourse.bass as bass
from concourse import mybir
from concourse.bass_utils import run_bass_kernel_spmd

B, D, C = 32768, 1024, 256
M_CORES = 8
BS = B // M_CORES
P = 128
GQ = 4
N_GROUPS = BS // (P * GQ)
N_SUB = N_GROUPS * GQ
WEIGHT = 0.0005
EPS = 1e-12

F32 = mybir.dt.float32
F32R = mybir.dt.float32r
BF16 = mybir.dt.bfloat16

XSLOTS = 7
XSS = 4
OHB = 8
OHF = 4
N_WARM = 16


def _sub_kind(t, a):
    return a == 3 and t % 2 == 0


def _ssq_on_dve(t, a):
    return a == 3


def build_nc(bs=BS):
    Sq = mybir.ActivationFunctionType.Square
    Sqrt = mybir.ActivationFunctionType.Sqrt
    CopyF = mybir.ActivationFunctionType.Copy

    LASTG = N_GROUPS - 1
    FINE_T = (0, 1, LASTG)

    def ssq_dve(t, a):
        return a == 3 or (t == LASTG and a == 2)

    dma_seq = [("aux",)]
    dma_seq += [("x", 0, a) for a in range(GQ)]
    dma_seq += [("x", 1, 0), ("x", 1, 1), ("lab",), ("x", 1, 2), ("x", 1, 3)]
    for t in range(2, N_GROUPS):
        if t in FINE_T:
            dma_seq += [("x", t, a) for a in range(GQ)]
        else:
            dma_seq += [("xg", t)]
    dma_idx = {}
    for i, e in enumerate(dma_seq):
        dma_idx[e] = (i + 1) * 16

    def x_wait(t, a):
        if t in FINE_T:
            return dma_idx[("x", t, a)]
        return dma_idx[("xg", t)]

    act_seq = []
    for t in range(N_GROUPS):
        if t == LASTG:
            for a in range(GQ):
                if not ssq_dve(t, a):
                    act_seq.append(("sq", t, a))
                    act_seq.append(("nrm", t, a))
            for a in range(GQ):
                if ssq_dve(t, a):
                    act_seq.append(("nrm", t, a))
        else:
            for a in range(GQ):
                if not ssq_dve(t, a):
                    act_seq.append(("sq", t, a))
            act_seq.append(("nrm", t))
        if t == 4:
            act_seq.append(("copyA", 0))
        if t == 5:
            act_seq.append(("copyA", 1))
    act_seq.append(("copyB",))
    act_idx = {e: i + 1 for i, e in enumerate(act_seq)}

    def nrm_wait(t, a):
        if t == LASTG:
            return act_idx[("nrm", t, a)]
        return act_idx[("nrm", t)]

    dve_seq = []
    for t in range(N_GROUPS):
        if t >= 1:
            tg = t - 1
            dve_seq.append(("recip", tg))
            dve_seq += [("oh", tg, a) for a in range(GQ)]
        if t == LASTG:
            dve_seq.append(("stt", t, 2))
            dve_seq.append(("stt", t, 3))
        else:
            dve_seq.append(("stt", t, 3))
    for a in range(GQ):
        dve_seq.append(("recip", LASTG, a))
        dve_seq.append(("oh", LASTG, a))
    dve_seq.append(("copyB",))
    dve_idx = {e: i + 1 for i, e in enumerate(dve_seq)}

    def oh_wait(t, a):
        if t == LASTG:
            return dve_idx[("oh", t, a)]
        return dve_idx[("oh", t, a)]

    def stt_wait(t, a):
        return dve_idx[("stt", t, a)]

    nc = bass.Bass()
    x = nc.declare_dram_parameter("x", [bs, D], F32R, isOutput=False)
    lab = nc.declare_dram_parameter("labf", [P, N_SUB], F32, isOutput=False)
    aux = nc.declare_dram_parameter("aux", [P, C + 2], F32R, isOutput=False)
    sumsA = nc.declare_dram_parameter("sumsA", [C, D], BF16, isOutput=True)
    sumsB = nc.declare_dram_parameter("sumsB", [C, D], BF16, isOutput=True)

    with ExitStack() as stk:
        en = stk.enter_context
        xt = en(nc.sbuf_tensor([P, XSLOTS, GQ, D], F32R))
        sqscr = en(nc.sbuf_tensor([P, D], BF16))
        vscr = en(nc.sbuf_tensor([P, D], BF16))
        auxs = en(nc.sbuf_tensor([P, C + 2], F32R))
        labf = en(nc.sbuf_tensor([P, N_SUB], F32))
        ssq = en(nc.sbuf_tensor([P, N_SUB], F32))
        nrm = en(nc.sbuf_tensor([P, N_SUB], F32))
        rr = en(nc.sbuf_tensor([P, N_SUB], F32))
        ohr = en(nc.sbuf_tensor([P, OHB, C], F32R))
        outA = en(nc.sbuf_tensor([P, 2, D], BF16))
        outB = en(nc.sbuf_tensor([P, 2, D], BF16))
        dum = en(nc.sbuf_tensor([P, 2], F32))
        ps = {}
        for h in range(2):
            for mi in range(2):
                ps[(h, mi)] = en(nc.psum_tensor(f"ps_{h}{mi}", [P, D], F32))

        s_in = en(nc.semaphore("s_in"))
        s_act = en(nc.semaphore("s_act"))
        s_dve = en(nc.semaphore("s_dve"))
        s_pe_mm = en(nc.semaphore("s_pe_mm"))
        s_out = en(nc.semaphore("s_out"))
        s_gp = en(nc.semaphore("s_gp"))
        block = en(nc.Block(no_gpsimd_drain=True))

        def x_src(t):
            return x[t * P * GQ : (t + 1) * P * GQ, :].rearrange(
                "(p g) d -> p g d", p=P
            )

        @block.sync
        def _(sync):
            sync.dma_start(out=auxs[:, :], in_=aux[:, :]).then_inc(s_in, 16)
            for e in dma_seq[1:]:
                if e[0] == "lab":
                    sync.dma_start(out=labf[:, :], in_=lab[:, :]).then_inc(
                        s_in, 16
                    )
                elif e[0] == "x":
                    _, t, a = e
                    if t >= XSLOTS:
                        sync.wait_ge(s_pe_mm, GQ * (t - XSLOTS + 1))
                    sync.dma_start(
                        out=xt[:, t % XSLOTS, a, :], in_=x_src(t)[:, a, :]
                    ).then_inc(s_in, 16)
                else:
                    _, t = e
                    if t >= XSLOTS:
                        sync.wait_ge(s_pe_mm, GQ * (t - XSLOTS + 1))
                    sync.dma_start(
                        out=xt[:, t % XSLOTS, :, :], in_=x_src(t)[:, :, :]
                    ).then_inc(s_in, 16)
            sync.wait_ge(s_act, act_idx[("copyA", 0)])
            sync.dma_start(out=sumsA[0:128, :], in_=outA[:, 0, :]).then_inc(
                s_out, 16
            )
            sync.wait_ge(s_act, act_idx[("copyA", 1)])
            sync.dma_start(out=sumsA[128:256, :], in_=outA[:, 1, :]).then_inc(
                s_out, 16
            )
            sync.wait_ge(s_act, act_idx[("copyB",)])
            sync.dma_start(out=sumsB[0:128, :], in_=outB[:, 0, :]).then_inc(
                s_out, 16
            )
            sync.wait_ge(s_dve, dve_idx[("copyB",)])
            sync.dma_start(out=sumsB[128:256, :], in_=outB[:, 1, :]).then_inc(
                s_out, 16
            )
            sync.wait_ge(s_out, 64)

        @block.scalar
        def _(scalar):
            scalar.activation(dum[:, 0:1], dum[:, 1:2], Sq, bias=0.0)
            scalar.activation(dum[:, 0:1], dum[:, 1:2], Sqrt, bias=0.0)
            scalar.wait_ge(s_in, dma_idx[("aux",)])
            eps_bias = auxs.bitcast(F32)[:, C + 1 : C + 2]
            n_sq = 0
            for e in act_seq:
                if e[0] == "sq":
                    _, t, a = e
                    k = t * GQ + a
                    scalar.wait_ge(s_in, x_wait(t, a))
                    scalar.activation(
                        sqscr[:, :],
                        xt.bitcast(F32)[:, t % XSLOTS, a, :],
                        Sq,
                        bias=0.0,
                        accum_out=ssq[:, k : k + 1],
                    ).then_inc(s_act, 1)
                    n_sq = act_idx[e]
                elif e[0] == "nrm":
                    if len(e) == 3:
                        _, t, a = e
                        if ssq_dve(t, a):
                            scalar.wait_ge(s_dve, stt_wait(t, a))
                        else:
                            scalar.wait_ge(s_act, n_sq)
                        k = t * GQ + a
                        scalar.activation(
                            nrm[:, k : k + 1], ssq[:, k : k + 1], Sqrt,
                            bias=eps_bias,
                        ).then_inc(s_act, 1)
                    else:
                        _, t = e
                        scalar.wait_ge(s_act, n_sq)
                        scalar.wait_ge(s_dve, stt_wait(t, 3))
                        scalar.activation(
                            nrm[:, t * GQ : (t + 1) * GQ],
                            ssq[:, t * GQ : (t + 1) * GQ],
                            Sqrt,
                            bias=eps_bias,
                        ).then_inc(s_act, 1)
                elif e[0] == "copyA":
                    _, mi = e
                    scalar.wait_ge(s_pe_mm, 16)
                    scalar.activation(
                        outA[:, mi, :], ps[(0, mi)][:, :], CopyF
                    ).then_inc(s_act, 1)
                else:
                    scalar.wait_ge(s_pe_mm, N_SUB)
                    scalar.activation(
                        outB[:, 0, :], ps[(1, 0)][:, :], CopyF
                    ).then_inc(s_act, 1)

        @block.vector
        def _(vector):
            vector.wait_ge(s_in, dma_idx[("lab",)])
            for e in dve_seq:
                if e[0] == "stt":
                    _, t, a = e
                    vector.wait_ge(s_in, x_wait(t, a))
                    k = t * GQ + a
                    xv = xt.bitcast(F32)[:, t % XSLOTS, a, :]
                    vector.scalar_tensor_tensor(
                        vscr[:, :], xv, 1.0, xv,
                        mybir.AluOpType.mult, mybir.AluOpType.mult,
                        accum_out=ssq[:, k : k + 1],
                    ).then_inc(s_dve, 1)
                elif e[0] == "recip":
                    if len(e) == 3:
                        _, tg, a = e
                        vector.wait_ge(s_act, nrm_wait(tg, a))
                        k = tg * GQ + a
                        vector.reciprocal(
                            rr[:, k : k + 1], nrm[:, k : k + 1]
                        ).then_inc(s_dve, 1)
                    else:
                        _, tg = e
                        vector.wait_ge(s_act, nrm_wait(tg, 0))
                        sl = slice(tg * GQ, (tg + 1) * GQ)
                        vector.reciprocal(rr[:, sl], nrm[:, sl]).then_inc(
                            s_dve, 1
                        )
                elif e[0] == "oh":
                    _, tg, a = e
                    k = tg * GQ + a
                    if a == 0 or tg == LASTG:
                        vector.wait_ge(s_dve, dve_idx[("recip", tg)
                                       if tg != LASTG else ("recip", tg, a)])
                    if k >= OHB and (a == 0 or tg == LASTG):
                        kl = tg * GQ + (GQ - 1 if tg != LASTG else a)
                        vector.wait_ge(s_pe_mm, kl - OHB + 1)
                    vector.tensor_scalar(
                        ohr[:, k % OHB, :],
                        auxs.bitcast(F32)[:, 0:C],
                        labf[:, k : k + 1],
                        rr[:, k : k + 1],
                        mybir.AluOpType.is_equal,
                        mybir.AluOpType.mult,
                    ).then_inc(s_dve, 1)
                else:
                    vector.wait_ge(s_pe_mm, N_SUB)
                    vector.tensor_copy(outB[:, 1, :], ps[(1, 1)][:, :]).then_inc(
                        s_dve, 1
                    )

        @block.tensor
        def _(tensor):
            tensor.wait_ge(s_in, dma_idx[("aux",)])
            for _ in range(N_WARM):
                tensor.matmul(
                    ps[(1, 0)][:, 0:256],
                    auxs[:, 0:128],
                    auxs[:, 0:256],
                    start=True,
                    stop=True,
                )
            for t in range(N_GROUPS):
                for a in range(GQ):
                    k = t * GQ + a
                    h = 0 if t < N_GROUPS // 2 else 1
                    first = k % 16 == 0
                    last = k % 16 == 15
                    tensor.wait_ge(s_dve, oh_wait(t, a))
                    tensor.wait_ge(s_in, x_wait(t, a))
                    w = ohr[:, k % OHB, :]
                    mv = xt[:, t % XSLOTS, a, :]
                    i = None
                    for mi in range(2):
                        for ni in range(2):
                            i = tensor.matmul(
                                ps[(h, mi)][:, ni * 512 : (ni + 1) * 512],
                                w[:, mi * 128 : (mi + 1) * 128],
                                mv[:, ni * 512 : (ni + 1) * 512],
                                start=first,
                                stop=last,
                            )
                    i.then_inc(s_pe_mm, 1)

    return nc


def _norm_rows(x):
    x = x.astype(np.float64)
    n = np.sqrt((x * x).sum(axis=-1, keepdims=True))
    return x / np.maximum(n, EPS)


def _host_finish(feats, labels, S):
    b, d = feats.shape
    counts = np.bincount(labels, minlength=C)
    n = counts.astype(np.float64)
    mask = n > 1.0
    normS2 = (S * S).sum(axis=1)
    term1 = float(((n - normS2 / np.maximum(n, 1.0)) * mask).sum())

    nc_of_row = counts[labels]
    rows = np.nonzero(np.arange(b) < nc_of_row)[0]
    corr = 0.0
    if rows.size:
        order = np.argsort(labels, kind="stable")
        cls_sorted = labels[order]
        starts = np.searchsorted(cls_sorted, np.arange(C))
        need = set()
        for i in rows:
            c = int(labels[i])
            if counts[c] <= 1:
                continue
            k = int(order[starts[c] + i])
            need.add(int(i))
            need.add(k)
        need = sorted(need)
        fcache = {i: _norm_rows(feats[i]) for i in need}
        for i in rows:
            c = int(labels[i])
            n_c = float(counts[c])
            if n_c <= 1.0:
                continue
            k = int(order[starts[c] + i])
            f_i = fcache[int(i)]
            f_k = fcache[k]
            Sc = S[c]
            c_simple = Sc / n_c
            c_true = (Sc - f_k) / (n_c - 1.0)
            d_true = float(((f_i - c_true) ** 2).sum())
            d_simple = float(((f_i - c_simple) ** 2).sum())
            corr += d_true - d_simple

    total = term1 + corr
    return np.array(WEIGHT * total / (b * d), dtype=np.float32)


_nc_cache = None

TRACE = False
LAST_RESULTS = None


def _round_fp32r(a):
    b = np.ascontiguousarray(a, dtype=np.float32).view(np.uint32)
    return ((b + 0x800) & 0xFFFFF000).view(np.float32)


def _aux_input():
    a = np.zeros((P, C + 2), dtype=np.float32)
    a[:, :C] = np.arange(C, dtype=np.float32)[None, :]
    a[:, C + 1] = 1e-20
    return _round_fp32r(a)


def kernel(features, labels):
    global _nc_cache, LAST_RESULTS
    feats = np.ascontiguousarray(np.asarray(features, dtype=np.float32))
    feats_r = _round_fp32r(feats)
    labs = np.ascontiguousarray(np.asarray(labels, dtype=np.int32))
    assert feats.shape == (B, D) and labs.shape == (B,)
    labs_f = labs.astype(np.float32)
    aux = _aux_input()
    if _nc_cache is None:
        _nc_cache = build_nc()
    in_maps = []
    for m in range(M_CORES):
        lf = labs_f[m * BS : (m + 1) * BS]
        lf = np.ascontiguousarray(
            lf.reshape(N_GROUPS, P, GQ).transpose(1, 0, 2).reshape(P, N_SUB)
        )
        in_maps.append(
            {"x": feats_r[m * BS : (m + 1) * BS], "labf": lf, "aux": aux}
        )
    res = run_bass_kernel_spmd(
        _nc_cache, in_maps, core_ids=list(range(M_CORES)), trace=TRACE
    )
    LAST_RESULTS = res
    S = np.zeros((C, D), np.float64)
    for r in res.results:
        S += r["sumsA"].astype(np.float64)
        S += r["sumsB"].astype(np.float64)
    return _host_finish(feats, labs, S)
